# revision 1
# baseline (speedup 1.0000x reference)
"""BiLSTM (nn_BiLSTM) Trainium2 Bass kernel — 8-core data-parallel on batch.

Per core (B_local = 32 of B = 256):
  - Gather the 6400 = T*32 embedding rows via indirect DMA into [128, 300+pad]
    tiles (4 timesteps per tile, t-major).
  - PE-transpose each tile's E-chunks -> xsT [E(part), rows], then matmul with
    W_x chunks to precompute xw^T = (x_t @ W_x + b)^T for all t, stored in
    SBUF per step/group/gate: xwT[:, t, grp, gate, b16]; the ACT copy out of
    PSUM folds the bias (incl. forget_bias).
  - Sequential forward LSTM over T=200 steps in transposed state layout.
    The batch is split into two independent groups of 16 whose dependency
    chains interleave on the engines, hiding cross-engine sync latency.
    Cell math uses sigmoid(x) = 0.5*tanh(x/2) + 0.5 with the 0.5-prescales
    folded into the weights and a doubled cell state C = 2c, H = 2h:
      psum_z = xwT_t (identity-matmul inject) + W_h-matmuls against H
      ACT:  tg = tanh(psum_z)            (one op, all 4 gates)
      DVE:  a = (tg_i + 1) * tg_j        (scalar_tensor_tensor)
            q = (tg_f + 1) * C
            C' = 0.5*q + a
      ACT:  tc = tanh(0.5 * C')
      DVE:  H' = (tg_o + 1) * tc
  - The "backward" direction of the reference only contributes bw_hs[0] =
    one LSTM step on x[:, T-1] from zero state, so no backward scan.
  - scores^T = (0.5*w_out)^T @ [H_fw; H_bw] + b_out -> [6, 32]; host
    transposes/concats.
Precompute is software-pipelined into the recurrence via interleaved emission.
"""

import ml_dtypes
import numpy as np

import concourse.bass as bass
import concourse.mybir as mybir
import concourse.tile as tile
from concourse import bacc
from concourse.bass_utils import run_bass_kernel_spmd

FP = mybir.dt.float32
BF = mybir.dt.bfloat16
I32 = mybir.dt.int32
WD = FP                     # dtype for matmul operands (weights/H/xwT/xsT)

# Problem constants
B, T_FULL, V, E, H, C = 256, 200, 50000, 300, 128, 6
NCORES = 8
BL = B // NCORES            # 32 rows per core
NG = 1                      # recurrence groups per core
GB = BL // NG               # 16 batch per group
TPB = 128 // BL             # 4 timesteps per gather tile
EC = 3                      # ceil(300/128) E chunks
EPAD = EC * 128             # 384
BLK = 4                     # gather tiles per stage-2 block
PERM = (1, 0, 2, 3)         # reference gate order (i,j,f,o) -> (j,i,f,o)
GJ, GI, GF, GO = 0, 1, 2, 3  # gate slots in permuted order
GSCALE = (1.0, 0.5, 0.5, 0.5)  # tanh prescale per gate (j full, i/f/o half)

Tanh = mybir.ActivationFunctionType.Tanh
Ident = mybir.ActivationFunctionType.Identity
MUL = mybir.AluOpType.mult
ADD = mybir.AluOpType.add


def build_nc(T=T_FULL, reps=1, variant="full", dynloop=False):
    RT = (T + TPB - 1) // TPB          # gather tiles (T=200 -> 50)
    assert RT * TPB == T
    NBLK = (RT + BLK - 1) // BLK       # stage-2 blocks (13)
    SPB = TPB * BLK                    # steps per full block (16)

    nc = bacc.Bacc("TRN2", target_bir_lowering=False, debug=False,
                   num_devices=NCORES)

    emb = nc.dram_tensor("emb", [V, E], FP, kind="ExternalInput")
    idx_d = nc.dram_tensor("idx", [128, RT], I32, kind="ExternalInput")
    wx_fw_d = nc.dram_tensor("wx_fw", [128, EC, 4 * H], WD, kind="ExternalInput")
    wh_fw_d = nc.dram_tensor("wh_fw", [128, 4 * H], WD, kind="ExternalInput")
    b_fw_d = nc.dram_tensor("b_fw", [128, 4], FP, kind="ExternalInput")
    wx_bw_d = nc.dram_tensor("wx_bw", [128, EC, 4 * H], WD, kind="ExternalInput")
    b_bw_d = nc.dram_tensor("b_bw", [128, 4], FP, kind="ExternalInput")
    wout_d = nc.dram_tensor("wout", [128, 2, C], WD, kind="ExternalInput")
    bout_d = nc.dram_tensor("bout", [C, 1], FP, kind="ExternalInput")
    ident_d = nc.dram_tensor("ident", [128, 128], FP, kind="ExternalInput")
    b4_d = nc.dram_tensor("b4", [4, 128], WD, kind="ExternalInput")
    sel_d = nc.dram_tensor("sel", [4, 512], WD, kind="ExternalInput")
    if dynloop:
        nrep_d = nc.dram_tensor("nrep", [1, 1], I32, kind="ExternalInput")
    out_d = nc.dram_tensor("out", [C, BL], FP, kind="ExternalOutput")

    with tile.TileContext(nc) as tc:
        with (
            tc.tile_pool(name="const", bufs=1) as cpool,
            tc.tile_pool(name="xs", bufs=10) as xs_pool,
            tc.tile_pool(name="xsT", bufs=2) as xsT_pool,
            tc.tile_pool(name="small", bufs=3) as sp,
            tc.tile_pool(name="pT", bufs=2, space="PSUM") as pT_pool,
            tc.tile_pool(name="ps2", bufs=6, space="PSUM") as ps2_pool,
        ):
            idx_sb = cpool.tile([128, RT], I32, tag="idx")
            wx_sb = cpool.tile([128, EC, 4 * H], WD, tag="wx")
            wh_sb = cpool.tile([128, 4 * H], WD, tag="wh")
            bf_sb = cpool.tile([128, 4], FP, tag="bf")
            wxb_sb = cpool.tile([128, EC, 4 * H], WD, tag="wxb")
            bb_sb = cpool.tile([128, 4], FP, tag="bb")
            wo_sb = cpool.tile([128, 2, C], WD, tag="wo")
            bo_sb = cpool.tile([C, 1], FP, tag="bo")
            id_sb = cpool.tile([128, 128], FP, tag="id")
            b4_sb = cpool.tile([4, 128], WD, tag="b4")
            sel_sb = cpool.tile([4, 512], WD, tag="sel")

            nc.sync.dma_start(out=idx_sb[:], in_=idx_d[:])
            nc.sync.dma_start(out=wx_sb[:], in_=wx_fw_d[:])
            nc.sync.dma_start(out=wh_sb[:], in_=wh_fw_d[:])
            nc.sync.dma_start(out=bf_sb[:], in_=b_fw_d[:])
            nc.sync.dma_start(out=wxb_sb[:], in_=wx_bw_d[:])
            nc.sync.dma_start(out=bb_sb[:], in_=b_bw_d[:])
            nc.sync.dma_start(out=wo_sb[:], in_=wout_d[:])
            nc.sync.dma_start(out=bo_sb[:], in_=bout_d[:])
            nc.sync.dma_start(out=id_sb[:], in_=ident_d[:])
            nc.sync.dma_start(out=b4_sb[:], in_=b4_d[:])
            nc.sync.dma_start(out=sel_sb[:], in_=sel_d[:])

            def emit_rep():
                xs_tiles = {}
                xsT_blks = {}
                zbanks = {}     # step//TPB -> psum bank [128, 4, TPB, BL]

                def zslice(t):
                    return zbanks[t // TPB][:, :, t % TPB, :]

                def blk_tiles(k):
                    return min(BLK, RT - k * BLK)

                def g_ops(q):
                    t_ = xs_pool.tile([128, EPAD], FP, tag="xs", name="xs")
                    xs_tiles[q] = t_
                    nc.gpsimd.indirect_dma_start(
                        out=t_[:, 0:E], out_offset=None,
                        in_=emb[:, :],
                        in_offset=bass.IndirectOffsetOnAxis(
                            ap=idx_sb[:, q:q + 1], axis=0),
                    )
                    nc.gpsimd.memset(t_[:, E:EPAD], 0.0)

                def t_ops(q):
                    k = q // BLK
                    if k not in xsT_blks:
                        xsT_blks[k] = xsT_pool.tile(
                            [128, EC, BLK * 128], WD, tag="xsT", name="xsT")
                    xst = xsT_blks[k]
                    xq = xs_tiles.pop(q)
                    pt = pT_pool.tile([128, EC, 128], FP, tag="pT", name="pT")
                    for e in range(EC):
                        nc.tensor.transpose(
                            pt[:, e, :], xq[:, e * 128:(e + 1) * 128], id_sb[:])
                    qq = q % BLK
                    nc.vector.tensor_copy(
                        xst[:, :, qq * 128:(qq + 1) * 128], pt[:])

                def s2_tile(q):
                    """x-side gates + bias for tile q (4 steps) into a z-bank.

                    Bank layout [128, gate, dt, b]; the bias lands via a
                    rank-4 matmul (lhsT=b4 [4,128], rhs=sel picks the gate
                    block per column). Gate h-matmuls accumulate later."""
                    k = q // BLK
                    xst = xsT_blks[k]
                    co = (q % BLK) * 128
                    zb = ps2_pool.tile([128, 4, TPB, BL], FP, tag="ps2",
                                       name="zb")
                    zbanks[q] = zb
                    flat = zb.rearrange("p g s b -> p (g s b)")
                    nc.tensor.matmul(flat[:, :], b4_sb[:], sel_sb[:],
                                     start=True, stop=False,
                                     skip_group_check=True)
                    for g in range(4):
                        for e in range(EC):
                            nc.tensor.matmul(
                                flat[:, g * 128:(g + 1) * 128],
                                wx_sb[:, e, g * H:(g + 1) * H],
                                xst[:, e, co:co + 128],
                                start=False, stop=False,
                                skip_group_check=True)

                # ---------------- prologue ----------------
                for q in range(min(2 * BLK, RT)):
                    g_ops(q)
                for q in range(min(BLK, RT)):
                    t_ops(q)
                for q in range(min(BLK, RT)):
                    s2_tile(q)
                for q in range(2 * BLK, min(3 * BLK, RT)):
                    g_ops(q)
                for q in range(BLK, min(2 * BLK, RT)):
                    t_ops(q)

                # ---------------- recurrence ----------------
                Hs = [None] * NG
                Cs = [None] * NG
                for t in range(T):
                    k, pos = t // SPB, t % SPB
                    if t > 0:
                        for n in range(NG):
                            for g in range(4):
                                nc.tensor.matmul(
                                    zbanks[t // TPB][:, g, t % TPB,
                                                     n * GB:(n + 1) * GB],
                                    wh_sb[:, g * H:(g + 1) * H],
                                    Hs[n][:],
                                    start=False,
                                    stop=(t % TPB == TPB - 1 and g == 3
                                          and n == NG - 1),
                                    skip_group_check=True)
                    if variant == "full":
                        # one background task per step, spread across the
                        # 4-step window: s2 / transposes / gather / idle
                        j, ph = pos // TPB, pos % TPB
                        if ph == 0:
                            q = (k + 1) * BLK + j
                            if q < RT:
                                s2_tile(q)
                        elif ph == 1:
                            q = (k + 2) * BLK + j
                            if q < RT:
                                t_ops(q)
                        elif ph == 2:
                            q = (k + 3) * BLK + j
                            if q < RT:
                                g_ops(q)
                    for n in range(NG):
                        zsl = zbanks[t // TPB][:, :, t % TPB,
                                               n * GB:(n + 1) * GB]
                        tg = sp.tile([128, 4, GB], FP, tag=f"tg{n}", name="tg")
                        nc.scalar.activation(tg[:], zsl, Tanh)
                        a = sp.tile([128, GB], FP, tag=f"a{n}", name="a")
                        nc.vector.scalar_tensor_tensor(
                            a[:], tg[:, GI, :], 1.0, tg[:, GJ, :], ADD, MUL)
                        if t == 0:
                            c_new = a
                        else:
                            q_ = sp.tile([128, GB], FP, tag=f"q{n}", name="q")
                            nc.vector.scalar_tensor_tensor(
                                q_[:], tg[:, GF, :], 1.0, Cs[n][:], ADD, MUL)
                            c_new = sp.tile([128, GB], FP, tag=f"c{n}",
                                            name="c")
                            nc.vector.scalar_tensor_tensor(
                                c_new[:], q_[:], 0.5, a[:], MUL, ADD)
                        Cs[n] = c_new
                        tc_ = sp.tile([128, GB], FP, tag=f"tc{n}", name="tc")
                        nc.scalar.activation(tc_[:], c_new[:], Tanh, scale=0.5)
                        h_new = sp.tile([128, GB], WD, tag=f"h{n}", name="h")
                        nc.vector.scalar_tensor_tensor(
                            h_new[:], tg[:, GO, :], 1.0, tc_[:], ADD, MUL)
                        Hs[n] = h_new
                    if t // TPB in zbanks and t % TPB == TPB - 1:
                        zbanks.pop(t // TPB)

                # ------------- backward: one step on x[:, T-1] -------------
                if variant == "full":
                    lastk = (RT - 1) // BLK
                    xst = xsT_blks[lastk]
                    coff = ((RT - 1) % BLK) * 128 + (TPB - 1) * BL
                    zbw = pT_pool.tile([128, 3 * BL], FP, tag="pT", name="zbw")
                    for s, g in ((0, GJ), (1, GI), (2, GO)):
                        for e in range(EC):
                            nc.tensor.matmul(
                                zbw[:, s * BL:(s + 1) * BL],
                                wxb_sb[:, e, g * H:(g + 1) * H],
                                xst[:, e, coff:coff + BL],
                                start=(s == 0 and e == 0),
                                stop=(s == 2 and e == EC - 1),
                                skip_group_check=True)
                    tjb = sp.tile([128, BL], FP, tag="tg0", name="tjb")
                    nc.scalar.activation(tjb[:], zbw[:, 0:BL], Tanh,
                                         bias=bb_sb[:, GJ:GJ + 1], scale=1.0)
                    tib = sp.tile([128, BL], FP, tag="tg1", name="tib")
                    nc.scalar.activation(tib[:], zbw[:, BL:2 * BL], Tanh,
                                         bias=bb_sb[:, GI:GI + 1], scale=0.5)
                    tob = sp.tile([128, BL], FP, tag="a0", name="tob")
                    nc.scalar.activation(tob[:], zbw[:, 2 * BL:3 * BL], Tanh,
                                         bias=bb_sb[:, GO:GO + 1], scale=0.5)
                    cbw = sp.tile([128, BL], FP, tag="c0", name="cbw")
                    nc.vector.scalar_tensor_tensor(
                        cbw[:], tib[:], 1.0, tjb[:], ADD, MUL)
                    tcb = sp.tile([128, BL], FP, tag="tc0", name="tcb")
                    nc.scalar.activation(tcb[:], cbw[:], Tanh, scale=0.5)
                    hbw = sp.tile([128, BL], WD, tag="h0", name="hbw")
                    nc.vector.scalar_tensor_tensor(
                        hbw[:], tob[:], 1.0, tcb[:], ADD, MUL)

                # ---------------- output ----------------
                po = pT_pool.tile([C, BL], FP, tag="pT", name="po")
                for n in range(NG):
                    nc.tensor.matmul(po[:, n * GB:(n + 1) * GB],
                                     wo_sb[:, 0, :], Hs[n][:],
                                     start=(n == 0),
                                     stop=(variant != "full" and n == NG - 1),
                                     skip_group_check=True)
                if variant == "full":
                    nc.tensor.matmul(po[:], wo_sb[:, 1, :], hbw[:],
                                     start=False, stop=True,
                                     skip_group_check=True)
                out_sb = sp.tile([C, BL], FP, tag="out", name="out")
                nc.scalar.activation(out_sb[:], po[:], Ident,
                                     bias=bo_sb[:, 0:1])
                nc.sync.dma_start(out=out_d[:], in_=out_sb[:])

            if dynloop:
                nrep_sb = cpool.tile([1, 1], I32, tag="nrep")
                nc.sync.dma_start(out=nrep_sb[:], in_=nrep_d[:])
                rv = nc.values_load(nrep_sb[0:1, 0:1], min_val=0, max_val=1024)
                with tc.For_i(0, rv, 1):
                    emit_rep()
            else:
                for _rep in range(reps):
                    emit_rep()

    nc.compile()
    return nc


# ---------------- host-side packing ----------------

def _wd_np():
    return ml_dtypes.bfloat16 if WD == BF else np.float32


def _permute_gates(w, scales=None):
    """Reorder trailing 4H axis (i,j,f,o)->(j,i,f,o), optionally scaling."""
    wg = w.reshape(*w.shape[:-1], 4, H)[..., PERM, :].copy()
    if scales is not None:
        for g in range(4):
            wg[..., g, :] *= scales[g]
    return wg.reshape(*w.shape)


def prep_inputs(x, embeds, W_fw, b_fw, W_bw, b_bw, w_out, b_out, T=T_FULL):
    RT = T // TPB
    x = np.asarray(x, np.int32)
    embeds = np.ascontiguousarray(np.asarray(embeds, np.float32))

    def pack_wx(W, scales):
        Wx = _permute_gates(np.asarray(W, np.float32)[:E], scales)
        pad = np.zeros((EPAD, 4 * H), np.float32)
        pad[:E] = Wx
        return np.ascontiguousarray(
            pad.reshape(EC, 128, 4 * H).transpose(1, 0, 2)
            .astype(_wd_np()))

    def pack_b(b_vec, forget_bias, scales):
        bg = np.asarray(b_vec, np.float32).reshape(4, H)[list(PERM)].copy()
        bg[GF] += forget_bias
        for g in range(4):
            bg[g] *= scales[g]
        return np.ascontiguousarray(bg.T)

    wh_scales = tuple(s * 0.5 for s in GSCALE)   # extra 0.5: H state = 2h
    shared = {
        "emb": embeds,
        "wx_fw": pack_wx(W_fw, GSCALE),
        "wh_fw": np.ascontiguousarray(
            _permute_gates(np.asarray(W_fw, np.float32)[E:], wh_scales)
            .astype(_wd_np())),
        "b_fw": pack_b(b_fw, 1.0, GSCALE),
        "wx_bw": pack_wx(W_bw, None),            # bw scales applied via ACT
        "b_bw": pack_b(b_bw, 1.0, GSCALE),
        "wout": np.ascontiguousarray(
            (0.5 * np.asarray(w_out, np.float32).reshape(2, H, C))
            .transpose(1, 0, 2).astype(_wd_np())),
        "bout": np.ascontiguousarray(np.asarray(b_out, np.float32)
                                     .reshape(C, 1)),
        "ident": np.eye(128, dtype=np.float32),
        "b4": np.ascontiguousarray(
            pack_b(b_fw, 1.0, GSCALE).T.astype(_wd_np())),
        "sel": np.ascontiguousarray(
            np.kron(np.eye(4), np.ones((1, 128))).astype(_wd_np())),
    }
    per_core = []
    for c in range(NCORES):
        xc = x[c * BL:(c + 1) * BL, :T]
        idxm = xc.T.reshape(RT, TPB, BL).reshape(RT, 128)
        per_core.append({"idx": np.ascontiguousarray(idxm.T), **shared})
    return per_core


_NC_CACHE = {}


def _get_nc(T=T_FULL):
    if T not in _NC_CACHE:
        _NC_CACHE[T] = build_nc(T)
    return _NC_CACHE[T]


def kernel(x, embeds, W_fw, b_fw, W_bw, b_bw, w_out, b_out):
    nc = _get_nc()
    in_maps = prep_inputs(x, embeds, W_fw, b_fw, W_bw, b_bw, w_out, b_out)
    res = run_bass_kernel_spmd(nc, in_maps, core_ids=list(range(NCORES)))
    out = np.empty((B, C), np.float32)
    for c in range(NCORES):
        out[c * BL:(c + 1) * BL] = res.results[c]["out"].T
    return out



# revision 7
# speedup vs baseline: 1.3211x; 1.3211x over previous
"""BiLSTM (nn_BiLSTM) Trainium2 Bass kernel — 8-core data-parallel on batch.

Per core (B_local = 32 of B = 256):
  - Gather the 6400 = T*32 embedding rows via indirect DMA into [128, 300+pad]
    tiles (4 timesteps per tile, t-major).
  - PE-transpose each tile's E-chunks -> xsT [E(part), rows], then matmul with
    W_x chunks (bf16) to precompute xw^T = (x_t @ W_x + b)^T for all t in
    PSUM z-banks [128, gate, TPB, BL]; a rank-4 matmul injects the bias.
  - Sequential forward LSTM over T=200 steps in transposed state layout,
    gate slot order (i, f, o, j) with tanh prescales (0.5, 0.5, 0.5, 1.0)
    folded into weights/bias, so one ACT op computes
      tg = tanh(z) for all 4 gates:  tg_i = 2sig(I)-1 etc., tg_j = tanh(J).
    Cell state is kept scaled: S = s*C' = 2s*c  (s = 0.2829..., chosen so the
    deg-7 tanh poly's leading Horner coefficient is exactly -1). Per step the
    cell math is 3 DVE ops (no second ACT visit, no extra engine hops):
      AQ_HALF (custom):  [a; q] = 0.5*(tg[i,f]+1) * [tg_j; S]
                          -> a = sig(I)tanh(J), q = sig(F+1)*S
      STT:               S' = (2s)*a + q
      LSTM_TAIL7 (custom, one 8-stage op):
                          H' = (tg_o+1) * S'*(P0 + y*(P1 + y*(P2 - y))), y=S'^2
                             = 2 sig(O) tanh(c')         [H = 2h convention]
  - The "backward" direction of the reference only contributes bw_hs[0] =
    one LSTM step on x[:, T-1] from zero state, so no backward scan.
  - scores^T = (0.5*w_out)^T @ [H_fw; H_bw] + b_out -> [6, 32]; host
    transposes/concats.
Precompute is software-pipelined into the recurrence via interleaved emission.
Matmul operands (W_x, W_h, xsT, H) are bf16; all accumulation is fp32.
"""

import ml_dtypes
import numpy as np

import concourse.bass as bass
import concourse.mybir as mybir
import concourse.tile as tile
from concourse import bacc
from concourse.bass_utils import run_bass_kernel_spmd

# ---------------- custom DVE op registration ----------------

import concourse.dve_ops as _dve_ops_mod
from concourse.dve_ops import DveOp as _DveOp
from concourse.dve_spec import (
    Spec as _Spec, Src0 as _Src0, Src1 as _Src1,
    C0 as _C0, C1 as _C1, C2 as _C2, One as _One,
    lower as _dve_lower, _has_src1 as _dve_has_src1,
)
from concourse.dve_uop import DveOpSpec as _DveOpSpec


def _register_dve_op(name, spec, subdim=False):
    for op in _dve_ops_mod.OPS:
        if op.name == name:
            return op
    op = _DveOp(name, spec, subdim, uops_sha={})
    row = max(_dve_ops_mod._SUB_OPCODE_FOR_NAME.values()) + 1
    assert row < 0x20, "custom-DVE opcode rows exhausted"
    _dve_ops_mod.OPS.append(op)
    _dve_ops_mod._SUB_OPCODE_FOR_NAME[name] = row
    _dve_ops_mod.CUSTOM_DVE_SPECS[name] = spec
    for ver in ("v3", "v4"):
        try:
            uops = _dve_lower(spec, ver=ver)
            op.uops_sha[ver] = _DveOpSpec(
                name=name, opcode=row, uops=uops, rd1_en=_dve_has_src1(spec)
            ).sha(ver)
        except Exception:
            pass
    return op


# [a; q; so] = (tg + 1) * in1 * c0 with c0 = 0.5; lanes (i,f,o) x (j,S,1)
# (STT shape: 2 free dims, so the 0.5 rides s0, not imm2)
AQ_HALF = _register_dve_op(
    "AQ_HALF_ANT",
    _Spec(
        body=(_Src0 + _One) * _Src1 * _C0,
        reference=lambda in0, in1, s0, s1, imm2: (
            (in0.astype(np.float32) + 1.0) * in1.astype(np.float32) * s0
        ),
    ),
)

# H' = (so * S) * (c0 + y*(c1 + y*(imm2 - y))), y = S^2 — sig(O) times the
# deg-7 scaled odd-poly 2*tanh(c') in one 8-stage op.
_y = _Src1 * _Src1
LSTM_TAIL7 = _register_dve_op(
    "LSTM_TAIL7_ANT",
    _Spec(
        body=(_Src0 * _Src1) * (_C0 + _y * (_C1 + _y * (_C2 - _y))),
        reference=lambda in0, in1, s0, s1, imm2: (
            (in0.astype(np.float32) * in1.astype(np.float32))
            * (s0 + (in1.astype(np.float32) ** 2)
               * (s1 + (in1.astype(np.float32) ** 2)
                  * (imm2 - in1.astype(np.float32) ** 2)))
        ),
    ),
)

FP = mybir.dt.float32
BF = mybir.dt.bfloat16
I32 = mybir.dt.int32
WD = BF                     # dtype for matmul operands (weights/H/xwT/xsT)

# Problem constants
B, T_FULL, V, E, H, C = 256, 200, 50000, 300, 128, 6
NCORES = 8
BL = B // NCORES            # 32 rows per core
GB = BL                     # recurrence batch per group (one group)
TPB = 128 // BL             # 4 timesteps per gather tile
EC = 3                      # ceil(300/128) E chunks
EPAD = EC * 128             # 384
BLK = 4                     # gather tiles per stage-2 block
# gate slot order (i, f, o, j): reference gate index per slot
PERM = (0, 2, 3, 1)
GI, GF, GO, GJ = 0, 1, 2, 3  # slots
GSCALE = (0.5, 0.5, 0.5, 1.0)  # tanh prescale per slot

# scaled-cell tanh tail constants: S = s*C', 2*tanh(C'/2) =
#   S*(P0 + y*(P1 + y*(P2 - y))), y = S^2   (minimax on |C'| <= 2.4)
S_SC = 0.3123542175171567
S2_SC = 2.0 * S_SC
TP0 = 3.1984743601072867
TP1 = -2.6540174491387427
TP2 = 2.198425447723119

Tanh = mybir.ActivationFunctionType.Tanh
Ident = mybir.ActivationFunctionType.Identity
MUL = mybir.AluOpType.mult
ADD = mybir.AluOpType.add


def build_nc(T=T_FULL, reps=1, variant="full", dynloop=False):
    RT = (T + TPB - 1) // TPB          # gather tiles (T=200 -> 50)
    assert RT * TPB == T
    NBLK = (RT + BLK - 1) // BLK       # stage-2 blocks (13)
    SPB = TPB * BLK                    # steps per full block (16)

    nc = bacc.Bacc("TRN2", target_bir_lowering=False, debug=False,
                   num_devices=NCORES)

    emb = nc.dram_tensor("emb", [V, E], FP, kind="ExternalInput")
    idx_d = nc.dram_tensor("idx", [128, RT], I32, kind="ExternalInput")
    wx_fw_d = nc.dram_tensor("wx_fw", [128, EC, 4 * H], WD, kind="ExternalInput")
    wh_fw_d = nc.dram_tensor("wh_fw", [128, 4 * H], WD, kind="ExternalInput")
    b_fw_d = nc.dram_tensor("b_fw", [128, 4], FP, kind="ExternalInput")
    wx_bw_d = nc.dram_tensor("wx_bw", [128, EC, 4 * H], WD, kind="ExternalInput")
    b_bw_d = nc.dram_tensor("b_bw", [128, 4], FP, kind="ExternalInput")
    wout_d = nc.dram_tensor("wout", [128, 2, C], WD, kind="ExternalInput")
    bout_d = nc.dram_tensor("bout", [C, 1], FP, kind="ExternalInput")
    ident_d = nc.dram_tensor("ident", [128, 128], FP, kind="ExternalInput")
    b4_d = nc.dram_tensor("b4", [4, 128], WD, kind="ExternalInput")
    sel_d = nc.dram_tensor("sel", [4, 512], WD, kind="ExternalInput")
    if dynloop:
        nrep_d = nc.dram_tensor("nrep", [1, 1], I32, kind="ExternalInput")
    out_d = nc.dram_tensor("out", [C, BL], FP, kind="ExternalOutput")

    with tile.TileContext(nc) as tc:
        with (
            tc.tile_pool(name="const", bufs=1) as cpool,
            tc.tile_pool(name="xs", bufs=10) as xs_pool,
            tc.tile_pool(name="xsT", bufs=2) as xsT_pool,
            tc.tile_pool(name="small", bufs=3) as sp,
            tc.tile_pool(name="pT", bufs=2, space="PSUM") as pT_pool,
            tc.tile_pool(name="ps2", bufs=6, space="PSUM") as ps2_pool,
        ):
            idx_sb = cpool.tile([128, RT], I32, tag="idx")
            wx_sb = cpool.tile([128, EC, 4 * H], WD, tag="wx")
            wh_sb = cpool.tile([128, 4 * H], WD, tag="wh")
            bf_sb = cpool.tile([128, 4], FP, tag="bf")
            wxb_sb = cpool.tile([128, EC, 4 * H], WD, tag="wxb")
            bb_sb = cpool.tile([128, 4], FP, tag="bb")
            wo_sb = cpool.tile([128, 2, C], WD, tag="wo")
            bo_sb = cpool.tile([C, 1], FP, tag="bo")
            id_sb = cpool.tile([128, 128], FP, tag="id")
            b4_sb = cpool.tile([4, 128], WD, tag="b4")
            sel_sb = cpool.tile([4, 512], WD, tag="sel")
            # ping-pong gate/state tiles: slots (i, f, o, j, S, ONE)
            t5 = [cpool.tile([128, 6, GB], FP, tag=f"t5{p}", name=f"t5{p}")
                  for p in range(2)]

            nc.sync.dma_start(out=idx_sb[:], in_=idx_d[:])
            nc.sync.dma_start(out=wx_sb[:], in_=wx_fw_d[:])
            nc.sync.dma_start(out=wh_sb[:], in_=wh_fw_d[:])
            nc.sync.dma_start(out=bf_sb[:], in_=b_fw_d[:])
            nc.sync.dma_start(out=wxb_sb[:], in_=wx_bw_d[:])
            nc.sync.dma_start(out=bb_sb[:], in_=b_bw_d[:])
            nc.sync.dma_start(out=wo_sb[:], in_=wout_d[:])
            nc.sync.dma_start(out=bo_sb[:], in_=bout_d[:])
            nc.sync.dma_start(out=id_sb[:], in_=ident_d[:])
            nc.sync.dma_start(out=b4_sb[:], in_=b4_d[:])
            nc.sync.dma_start(out=sel_sb[:], in_=sel_d[:])

            def emit_rep():
                xs_tiles = {}
                xsT_blks = {}
                zbanks = {}     # step//TPB -> psum bank [128, 4, TPB, BL]

                nc.gpsimd.memset(t5[0][:, 4, :], 0.0)
                nc.gpsimd.memset(t5[0][:, 5, :], 1.0)
                nc.gpsimd.memset(t5[1][:, 5, :], 1.0)

                def blk_tiles(k):
                    return min(BLK, RT - k * BLK)

                def g_ops(q):
                    t_ = xs_pool.tile([128, EPAD], FP, tag="xs", name="xs")
                    xs_tiles[q] = t_
                    nc.gpsimd.indirect_dma_start(
                        out=t_[:, 0:E], out_offset=None,
                        in_=emb[:, :],
                        in_offset=bass.IndirectOffsetOnAxis(
                            ap=idx_sb[:, q:q + 1], axis=0),
                    )
                    nc.gpsimd.memset(t_[:, E:EPAD], 0.0)

                def t_ops(q):
                    k = q // BLK
                    if k not in xsT_blks:
                        xsT_blks[k] = xsT_pool.tile(
                            [128, EC, BLK * 128], WD, tag="xsT", name="xsT")
                    xst = xsT_blks[k]
                    xq = xs_tiles.pop(q)
                    pt = pT_pool.tile([128, EC, 128], FP, tag="pT", name="pT")
                    for e in range(EC):
                        nc.tensor.transpose(
                            pt[:, e, :], xq[:, e * 128:(e + 1) * 128], id_sb[:])
                    qq = q % BLK
                    nc.vector.tensor_copy(
                        xst[:, :, qq * 128:(qq + 1) * 128], pt[:])

                def s2_tile(q):
                    """x-side gates + bias for tile q (4 steps) into a z-bank.

                    Bank layout [128, gate, dt, b]; the bias lands via a
                    rank-4 matmul (lhsT=b4 [4,128], rhs=sel picks the gate
                    block per column). Gate h-matmuls accumulate later."""
                    k = q // BLK
                    xst = xsT_blks[k]
                    co = (q % BLK) * 128
                    zb = ps2_pool.tile([128, 4, TPB, BL], FP, tag="ps2",
                                       name="zb")
                    zbanks[q] = zb
                    flat = zb.rearrange("p g s b -> p (g s b)")
                    nc.tensor.matmul(flat[:, :], b4_sb[:], sel_sb[:],
                                     start=True, stop=False,
                                     skip_group_check=True)
                    for g in range(4):
                        for e in range(EC):
                            nc.tensor.matmul(
                                flat[:, g * 128:(g + 1) * 128],
                                wx_sb[:, e, g * H:(g + 1) * H],
                                xst[:, e, co:co + 128],
                                start=False, stop=False,
                                skip_group_check=True)

                # ---------------- prologue ----------------
                for q in range(min(2 * BLK, RT)):
                    g_ops(q)
                for q in range(min(BLK, RT)):
                    t_ops(q)
                for q in range(min(BLK, RT)):
                    s2_tile(q)
                for q in range(2 * BLK, min(3 * BLK, RT)):
                    g_ops(q)
                for q in range(BLK, min(2 * BLK, RT)):
                    t_ops(q)

                # ---------------- recurrence ----------------
                Hcur = None
                for t in range(T):
                    k, pos = t // SPB, t % SPB
                    cur = t5[t % 2]
                    nxt = t5[(t + 1) % 2]
                    if t > 0:
                        for g in range(4):
                            nc.tensor.matmul(
                                zbanks[t // TPB][:, g, t % TPB, :],
                                wh_sb[:, g * H:(g + 1) * H],
                                Hcur[:],
                                start=False,
                                stop=(t % TPB == TPB - 1 and g == 3),
                                skip_group_check=True)
                    if variant == "full":
                        # one background task per step, spread across the
                        # 4-step window: s2 / transposes / gather / idle
                        j, ph = pos // TPB, pos % TPB
                        if ph == 0:
                            q = (k + 1) * BLK + j
                            if q < RT:
                                s2_tile(q)
                        elif ph == 1:
                            q = (k + 2) * BLK + j
                            if q < RT:
                                t_ops(q)
                        elif ph == 2:
                            q = (k + 3) * BLK + j
                            if q < RT:
                                g_ops(q)
                    zsl = zbanks[t // TPB][:, :, t % TPB, :]
                    nc.scalar.activation(cur[:, 0:4, :], zsl, Tanh)
                    aq = sp.tile([128, 3, GB], FP, tag="aq", name="aq")
                    nc.vector._custom_dve(
                        AQ_HALF, out=aq[:],
                        in0=cur[:, 0:3, :], in1=cur[:, 3:6, :], s0=0.5,
                    )
                    nc.vector.scalar_tensor_tensor(
                        nxt[:, 4, :], aq[:, 0, :], S2_SC, aq[:, 1, :],
                        MUL, ADD)
                    h_new = sp.tile([128, GB], WD, tag="h", name="h")
                    nc.vector._custom_dve(
                        LSTM_TAIL7, out=h_new[:],
                        in0=aq[:, 2, :], in1=nxt[:, 4, :],
                        s0=TP0, s1=TP1, imm2=TP2,
                    )
                    Hcur = h_new
                    if t // TPB in zbanks and t % TPB == TPB - 1:
                        zbanks.pop(t // TPB)

                # ------------- backward: one step on x[:, T-1] -------------
                if variant == "full":
                    lastk = (RT - 1) // BLK
                    xst = xsT_blks[lastk]
                    coff = ((RT - 1) % BLK) * 128 + (TPB - 1) * BL
                    zbw = pT_pool.tile([128, 3 * BL], FP, tag="pT", name="zbw")
                    for s, g in ((0, GJ), (1, GI), (2, GO)):
                        for e in range(EC):
                            nc.tensor.matmul(
                                zbw[:, s * BL:(s + 1) * BL],
                                wxb_sb[:, e, g * H:(g + 1) * H],
                                xst[:, e, coff:coff + BL],
                                start=(s == 0 and e == 0),
                                stop=(s == 2 and e == EC - 1),
                                skip_group_check=True)
                    tjb = sp.tile([128, BL], FP, tag="tg0", name="tjb")
                    nc.scalar.activation(tjb[:], zbw[:, 0:BL], Tanh,
                                         bias=bb_sb[:, GJ:GJ + 1], scale=1.0)
                    tib = sp.tile([128, BL], FP, tag="tg1", name="tib")
                    nc.scalar.activation(tib[:], zbw[:, BL:2 * BL], Tanh,
                                         bias=bb_sb[:, GI:GI + 1], scale=0.5)
                    tob = sp.tile([128, BL], FP, tag="aq", name="tob")
                    nc.scalar.activation(tob[:], zbw[:, 2 * BL:3 * BL], Tanh,
                                         bias=bb_sb[:, GO:GO + 1], scale=0.5)
                    cbw = sp.tile([128, BL], FP, tag="c0", name="cbw")
                    nc.vector.scalar_tensor_tensor(
                        cbw[:], tib[:], 1.0, tjb[:], ADD, MUL)
                    tcb = sp.tile([128, BL], FP, tag="tc0", name="tcb")
                    nc.scalar.activation(tcb[:], cbw[:], Tanh, scale=0.5)
                    hbw = sp.tile([128, BL], WD, tag="h0", name="hbw")
                    nc.vector.scalar_tensor_tensor(
                        hbw[:], tob[:], 1.0, tcb[:], ADD, MUL)

                # ---------------- output ----------------
                po = pT_pool.tile([C, BL], FP, tag="pT", name="po")
                nc.tensor.matmul(po[:], wo_sb[:, 0, :], Hcur[:],
                                 start=True,
                                 stop=(variant != "full"),
                                 skip_group_check=True)
                if variant == "full":
                    nc.tensor.matmul(po[:], wo_sb[:, 1, :], hbw[:],
                                     start=False, stop=True,
                                     skip_group_check=True)
                out_sb = sp.tile([C, BL], FP, tag="out", name="out")
                nc.scalar.activation(out_sb[:], po[:], Ident,
                                     bias=bo_sb[:, 0:1])
                nc.sync.dma_start(out=out_d[:], in_=out_sb[:])

            if dynloop:
                nrep_sb = cpool.tile([1, 1], I32, tag="nrep")
                nc.sync.dma_start(out=nrep_sb[:], in_=nrep_d[:])
                rv = nc.values_load(nrep_sb[0:1, 0:1], min_val=0, max_val=1024)
                with tc.For_i(0, rv, 1):
                    emit_rep()
            else:
                for _rep in range(reps):
                    emit_rep()

    nc.compile()
    return nc


# ---------------- host-side packing ----------------

def _wd_np():
    return ml_dtypes.bfloat16 if WD == BF else np.float32


def _permute_gates(w, scales=None):
    """Reorder trailing 4H axis to slot order (i,f,o,j), optionally scaling."""
    wg = w.reshape(*w.shape[:-1], 4, H)[..., PERM, :].copy()
    if scales is not None:
        for g in range(4):
            wg[..., g, :] *= scales[g]
    return wg.reshape(*w.shape)


def prep_inputs(x, embeds, W_fw, b_fw, W_bw, b_bw, w_out, b_out, T=T_FULL):
    RT = T // TPB
    x = np.asarray(x, np.int32)
    embeds = np.ascontiguousarray(np.asarray(embeds, np.float32))

    def pack_wx(W, scales):
        Wx = _permute_gates(np.asarray(W, np.float32)[:E], scales)
        pad = np.zeros((EPAD, 4 * H), np.float32)
        pad[:E] = Wx
        return np.ascontiguousarray(
            pad.reshape(EC, 128, 4 * H).transpose(1, 0, 2)
            .astype(_wd_np()))

    def pack_b(b_vec, forget_bias, scales):
        bg = np.asarray(b_vec, np.float32).reshape(4, H)[list(PERM)].copy()
        bg[GF] += forget_bias
        for g in range(4):
            bg[g] *= scales[g]
        return np.ascontiguousarray(bg.T)

    wh_scales = tuple(s * 0.5 for s in GSCALE)   # extra 0.5: H state = 2h
    shared = {
        "emb": embeds,
        "wx_fw": pack_wx(W_fw, GSCALE),
        "wh_fw": np.ascontiguousarray(
            _permute_gates(np.asarray(W_fw, np.float32)[E:], wh_scales)
            .astype(_wd_np())),
        "b_fw": pack_b(b_fw, 1.0, GSCALE),
        "wx_bw": pack_wx(W_bw, None),            # bw scales applied via ACT
        "b_bw": pack_b(b_bw, 1.0, GSCALE),
        "wout": np.ascontiguousarray(
            (0.5 * np.asarray(w_out, np.float32).reshape(2, H, C))
            .transpose(1, 0, 2).astype(_wd_np())),
        "bout": np.ascontiguousarray(np.asarray(b_out, np.float32)
                                     .reshape(C, 1)),
        "ident": np.eye(128, dtype=np.float32),
        "b4": np.ascontiguousarray(
            pack_b(b_fw, 1.0, GSCALE).T.astype(_wd_np())),
        "sel": np.ascontiguousarray(
            np.kron(np.eye(4), np.ones((1, 128))).astype(_wd_np())),
    }
    per_core = []
    for c in range(NCORES):
        xc = x[c * BL:(c + 1) * BL, :T]
        idxm = xc.T.reshape(RT, TPB, BL).reshape(RT, 128)
        per_core.append({"idx": np.ascontiguousarray(idxm.T), **shared})
    return per_core


_NC_CACHE = {}


def _get_nc(T=T_FULL):
    if T not in _NC_CACHE:
        _NC_CACHE[T] = build_nc(T)
    return _NC_CACHE[T]


def kernel(x, embeds, W_fw, b_fw, W_bw, b_bw, w_out, b_out):
    nc = _get_nc()
    in_maps = prep_inputs(x, embeds, W_fw, b_fw, W_bw, b_bw, w_out, b_out)
    res = run_bass_kernel_spmd(nc, in_maps, core_ids=list(range(NCORES)))
    out = np.empty((B, C), np.float32)
    for c in range(NCORES):
        out[c * BL:(c + 1) * BL] = res.results[c]["out"].T
    return out
